# revision 56
# baseline (speedup 1.0000x reference)
"""Bahdanau attention Trainium2 kernel (nn_BahdanauAttn_52862457479409).

Shapes (hardcoded): B=64, S=4096, H=128, D=2H=256, f32.
Sharding: data-parallel over batch, 8 batches per core on 8 cores.

Per-core per-batch pipeline (SPMD over cores, software-pipelined so
phase B of batch b is emitted after the score-front of batch b+1):
  1. enc[b] (4 MB) loaded HBM->SBUF as float32r natural layout
     nat[p=s%128, c=s//128, d] in 16 split DMAs on the pure SP queue
     (the f32r-typed DMA delivers hardware-rounded data, which the
     fp32r matmul path requires anyway).
  2. PE-transposes (f32r, 1.5 cy/row) -> encT tiles [128d, 512s];
     PSUM->SBUF copies balanced ~11/5 across DVE/ACT.
  3. Phase A: enc_featT = WhT_r.T @ encT (f32r, N=512, full rate).
  4. ACT tanh with per-partition bias dec_featT[:, b] -> T tiles (f32).
  5. v-matmuls (fp32, exact) batched at front end: scoresT[128s, 32c]
     in PSUM - softmax over all 4096 is lane-parallel in this layout;
     max-subtraction is safely skipped (|scores| <= ~6).
  6. exp (ACT) + fused mask-select/row-sum (DVE scalar_tensor_tensor)
     -> em; cross-partition total + reciprocal broadcast via tiny PE
     ones-matmuls.
  7. attn output: PE transpose of normalized attnT -> [32, 128] rows.
  8. Phase B on UNNORMALIZED weights: 32 f32r matmuls with em column
     pairs stationary (padded to 2 cols for the even-count ISA rule,
     garbage row 1 ignored), rhs = nat chunks N=256; context scaled by
     1/total at the end and stored as one contiguous row.

Only the fp32r rounding (~11 mantissa bits) of enc/Wh/em leaves the
fp32 envelope; measured ~1.3e-4 relative error on both outputs.
TimelineSim: ~128 us/core against a ~97 us DMA roofline.
"""

from contextlib import ExitStack

import numpy as np

import concourse.bass as bass
import concourse.tile as tile
from concourse import bacc, mybir
from concourse.bass_utils import run_bass_kernel_spmd
from concourse.masks import make_identity

B, S, H = 64, 4096, 128
D = 2 * H
N_CORES = 8
BPC = B // N_CORES  # batches per core
C = S // 128  # 32 s-chunks of 128
NT = 8  # s-tiles per batch
TS = S // NT  # 512, s-tile size
CPT = TS // 128  # 4 chunks per tile

_CACHE = {}


def _build():
    if "nc" in _CACHE:
        return _CACHE["nc"]
    dt = mybir.dt
    nc = bacc.Bacc("TRN2", target_bir_lowering=False, debug=False, num_devices=N_CORES)

    enc = nc.dram_tensor("enc", [BPC, S, D], dt.float32, kind="ExternalInput").ap()
    msk = nc.dram_tensor("msk", [BPC, S], dt.uint8, kind="ExternalInput").ap()
    hid = nc.dram_tensor("hid", [BPC, H], dt.float32, kind="ExternalInput").ap()
    cel = nc.dram_tensor("cel", [BPC, H], dt.float32, kind="ExternalInput").ap()
    wh = nc.dram_tensor("wh", [H, D], dt.float32, kind="ExternalInput").ap()
    ws = nc.dram_tensor("ws", [H, D], dt.float32, kind="ExternalInput").ap()
    wsb = nc.dram_tensor("wsb", [H], dt.float32, kind="ExternalInput").ap()
    vw = nc.dram_tensor("vw", [1, H], dt.float32, kind="ExternalInput").ap()
    ctx_o = nc.dram_tensor("ctx", [BPC, D], dt.float32, kind="ExternalOutput").ap()
    attn_o = nc.dram_tensor("attn", [BPC, S], dt.float32, kind="ExternalOutput").ap()

    with tile.TileContext(nc) as tc, ExitStack() as ctx:
        singles = ctx.enter_context(tc.tile_pool(name="singles", bufs=1))
        natp = ctx.enter_context(tc.tile_pool(name="nat", bufs=3))
        etp = ctx.enter_context(tc.tile_pool(name="encT", bufs=4))
        ttp = ctx.enter_context(tc.tile_pool(name="tanh", bufs=12))
        smallp = ctx.enter_context(tc.tile_pool(name="small", bufs=4))
        # PSUM pools: 2 + 2 + 1 + 2 + 1 = 8 banks
        tpp = ctx.enter_context(tc.tile_pool(name="tp_ps", bufs=4, space="PSUM"))
        efp = ctx.enter_context(tc.tile_pool(name="ef_ps", bufs=1, space="PSUM"))
        scp = ctx.enter_context(tc.tile_pool(name="sc_ps", bufs=1, space="PSUM"))
        cxp = ctx.enter_context(tc.tile_pool(name="cx_ps", bufs=1, space="PSUM"))
        mip = ctx.enter_context(tc.tile_pool(name="mi_ps", bufs=1, space="PSUM"))

        ident = singles.tile([128, 128], dt.float32)
        make_identity(nc, ident[:])
        ident_r = singles.tile([128, 128], dt.float32r)
        nc.vector.tensor_copy(ident_r[:], ident[:])
        ones = singles.tile([128, 1], dt.float32)
        nc.vector.memset(ones[:], 1.0)
        ones_row = singles.tile([1, 128], dt.float32)
        nc.vector.memset(ones_row[:], 1.0)
        zeros2 = singles.tile([128, 2], dt.float32)
        nc.vector.memset(zeros2[:], 0.0)

        # ---- setup: WhT (f32r), WsT (f32), v column (f32r), dec_featT ----
        swh = singles.tile([H, D], dt.float32)
        nc.sync.dma_start(swh[:], wh[:])
        sws = singles.tile([H, D], dt.float32)
        nc.sync.dma_start(sws[:], ws[:])
        whT_r = singles.tile([128, 2, 128], dt.float32r)
        wsT = singles.tile([128, 2, 128], dt.float32)
        for half in range(2):
            pt = tpp.tile([128, 128], dt.float32, tag="tp")
            nc.tensor.transpose(pt[:], swh[:, bass.ts(half, 128)], ident[:])
            nc.vector.tensor_copy(whT_r[:, half, :], pt[:])
            pt2 = tpp.tile([128, 128], dt.float32, tag="tp")
            nc.tensor.transpose(pt2[:], sws[:, bass.ts(half, 128)], ident[:])
            nc.vector.tensor_copy(wsT[:, half, :], pt2[:])

        # v row -> column (PE transpose), rounded to f32r
        sv = singles.tile([1, H], dt.float32)
        nc.sync.dma_start(sv[:], vw[:])
        pv = mip.tile([128, 1], dt.float32, tag="mi")
        nc.tensor.transpose(pv[:], sv[:], ident[:1, :1])
        v_r = singles.tile([128, 1], dt.float32)
        nc.vector.tensor_copy(v_r[:], pv[:])

        # dec_featT [128h, BPC] = WsT.T @ [hidT; celT] + wsb
        shid = singles.tile([BPC, H], dt.float32)
        nc.sync.dma_start(shid[:], hid[:])
        scel = singles.tile([BPC, H], dt.float32)
        nc.sync.dma_start(scel[:], cel[:])
        sbias = singles.tile([H, 1], dt.float32)
        nc.sync.dma_start(sbias[:], wsb[:, None])
        hidT = singles.tile([H, BPC], dt.float32)
        celT = singles.tile([H, BPC], dt.float32)
        ph = mip.tile([H, BPC], dt.float32, tag="mi")
        nc.tensor.transpose(ph[:], shid[:], ident[:BPC, :BPC])
        nc.vector.tensor_copy(hidT[:], ph[:])
        pc2 = mip.tile([H, BPC], dt.float32, tag="mi")
        nc.tensor.transpose(pc2[:], scel[:], ident[:BPC, :BPC])
        nc.vector.tensor_copy(celT[:], pc2[:])
        pdf = mip.tile([H, BPC], dt.float32, tag="mi")
        nc.tensor.matmul(pdf[:], wsT[:, 0, :], hidT[:], start=True, stop=False)
        nc.tensor.matmul(pdf[:], wsT[:, 1, :], celT[:], start=False, stop=True)
        dec_featT = singles.tile([H, BPC], dt.float32)
        nc.scalar.activation(
            dec_featT[:], pdf[:], mybir.ActivationFunctionType.Identity, bias=sbias[:]
        )

        # ---- per-batch pipeline (software-pipelined emission) ----
        # front(b): DMA + transposes + phase A + tanh + v + softmax -> attnT
        # phaseB(b) is emitted AFTER front(b+1) so the in-order PE stream
        # always has ready work while batch b's softmax chain runs on DVE/ACT.
        state = {}

        def front(b):
            nat = natp.tile([128, C, D], dt.float32r)
            enc_b = enc[b].rearrange("(c p) d -> p c d", p=128).bitcast(dt.float32r)
            for k in range(16):
                nc.sync.dma_start(
                    nat[:, bass.ts(k, C // 16), :], enc_b[:, bass.ts(k, C // 16), :]
                )

            # mask [32, 128] natural -> f32 -> maskT [128, 32]
            mnat = smallp.tile([C, 128], dt.uint8)
            nc.gpsimd.dma_start(mnat[:], msk[b].rearrange("(c p) -> c p", p=128))
            mnat_f = smallp.tile([C, 128], dt.float32)
            nc.vector.tensor_copy(mnat_f[:], mnat[:])
            pmt = mip.tile([128, C], dt.float32, tag="mi")
            nc.tensor.transpose(pmt[:], mnat_f[:], ident[:C, :C])
            maskT = smallp.tile([128, C], dt.float32)
            nc.vector.tensor_copy(maskT[:], pmt[:])

            scoresT = scp.tile([128, C], dt.float32)
            tts = []
            for t in range(NT):
                eT0 = etp.tile([128, TS], dt.float32r, tag="eT0")
                eT1 = etp.tile([128, TS], dt.float32r, tag="eT1")
                for half, eT in ((0, eT0), (1, eT1)):
                    tp = tpp.tile([128, TS], dt.float32r, tag="tp")
                    for j in range(CPT):
                        ch = CPT * t + j
                        nc.tensor.transpose(
                            tp[:, bass.ts(j, 128)],
                            nat[:, ch, bass.ts(half, 128)],
                            ident_r[:],
                        )
                    # PSUM->SBUF copy; balance ~11 on DVE, ~5 on ACT
                    if (2 * t + half) % 3 == 2:
                        nc.scalar.copy(eT[:], tp[:])
                    else:
                        nc.vector.tensor_copy(eT[:], tp[:])
                ef = efp.tile([128, TS], dt.float32)
                nc.tensor.matmul(ef[:], whT_r[:, 0, :], eT0[:], start=True, stop=False)
                nc.tensor.matmul(ef[:], whT_r[:, 1, :], eT1[:], start=False, stop=True)
                tt = ttp.tile([128, TS], dt.float32)
                nc.scalar.activation(
                    tt[:],
                    ef[:],
                    mybir.ActivationFunctionType.Tanh,
                    bias=dec_featT[:, b : b + 1],
                )
                tts.append(tt)
            for t in range(NT):
                tt = tts[t]
                for j in range(CPT):
                    nc.tensor.matmul(
                        scoresT[:, CPT * t + j : CPT * t + j + 1],
                        tt[:, bass.ts(j, 128)],
                        v_r[:],
                        start=True,
                        stop=True,
                    )

            # softmax over all 4096 (no max-subtraction: |scores| <= ~6)
            expT = smallp.tile([128, C], dt.float32)
            nc.scalar.activation(expT[:], scoresT[:], mybir.ActivationFunctionType.Exp)
            em = smallp.tile([128, C], dt.float32)
            partial = smallp.tile([128, 1], dt.float32)
            # em = (maskT == 0) * expT ; partial = row-sum(em)
            nc.vector.scalar_tensor_tensor(
                out=em[:],
                in0=maskT[:],
                scalar=0.0,
                in1=expT[:],
                op0=mybir.AluOpType.is_equal,
                op1=mybir.AluOpType.mult,
                accum_out=partial[:],
            )
            # phase B runs on UNNORMALIZED weights (em); ctx scaled at end.
            em_r = smallp.tile([128, C + 2], dt.float32r)
            nc.vector.tensor_copy(em_r[:, :C], em[:])
            nc.vector.tensor_copy(em_r[:, C : C + 2], zeros2[:])
            ptot = mip.tile([1, 1], dt.float32, tag="mi")
            nc.tensor.matmul(ptot[:], partial[:], ones[:], start=True, stop=True)
            recip = smallp.tile([1, 1], dt.float32)
            nc.vector.reciprocal(recip[:], ptot[:])
            recip_bc = mip.tile([128, 1], dt.float32, tag="mi")
            nc.tensor.matmul(recip_bc[:], ones_row[:], recip[:], start=True, stop=True)
            attnT = smallp.tile([128, C], dt.float32)
            nc.vector.tensor_scalar_mul(attnT[:], em[:], recip_bc[:])
            state[b] = (nat, attnT, em_r, recip)

        def phase_b(b):
            nat, attnT, em_r, recip = state.pop(b)
            # attn output: transpose -> [32, 128] natural rows
            pat = mip.tile([C, 128], dt.float32, tag="mi")
            nc.tensor.transpose(pat[:], attnT[:], ident[:])
            at_sb = smallp.tile([C, 128], dt.float32)
            nc.vector.tensor_copy(at_sb[:], pat[:])
            nc.gpsimd.dma_start(attn_o[b].rearrange("(c p) -> c p", p=128), at_sb[:])

            # phase B: context via attn-stationary f32r matmuls (row 1 garbage)
            cx = cxp.tile([2, D], dt.float32, tag="cx")
            for c in range(C):
                nc.tensor.matmul(
                    cx[:],
                    em_r[:, c : c + 2],
                    nat[:, c, :],
                    start=(c == 0),
                    stop=(c == C - 1),
                )
            ctx_sb = smallp.tile([1, D], dt.float32)
            nc.vector.tensor_scalar_mul(ctx_sb[:], cx[0:1, :], recip[:])
            nc.gpsimd.dma_start(ctx_o[b : b + 1, :], ctx_sb[:])

        for b in range(BPC):
            front(b)
            if b > 0:
                phase_b(b - 1)
        phase_b(BPC - 1)

    nc.compile()
    _CACHE["nc"] = nc
    return nc


def kernel(
    encoder_outputs,
    encoder_padding_mask,
    hidden_state,
    cell_state,
    Wh_w,
    Ws_w,
    Ws_b,
    v_w,
):
    nc = _build()
    enc = np.ascontiguousarray(np.asarray(encoder_outputs, dtype=np.float32))
    mask = np.asarray(encoder_padding_mask).astype(np.uint8)
    hid = np.asarray(hidden_state, dtype=np.float32)
    cel = np.asarray(cell_state, dtype=np.float32)
    wh = np.ascontiguousarray(np.asarray(Wh_w, dtype=np.float32))
    ws = np.ascontiguousarray(np.asarray(Ws_w, dtype=np.float32))
    wsb = np.asarray(Ws_b, dtype=np.float32)
    vw = np.ascontiguousarray(np.asarray(v_w, dtype=np.float32))

    in_maps = []
    for c in range(N_CORES):
        sl = slice(c * BPC, (c + 1) * BPC)
        in_maps.append(
            {
                "enc": enc[sl],
                "msk": mask[sl],
                "hid": hid[sl],
                "cel": cel[sl],
                "wh": wh,
                "ws": ws,
                "wsb": wsb,
                "vw": vw,
            }
        )
    res = run_bass_kernel_spmd(nc, in_maps, core_ids=list(range(N_CORES)))
    context = np.concatenate([r["ctx"] for r in res.results], axis=0)
    attn = np.concatenate([r["attn"] for r in res.results], axis=0)
    return context, attn


# revision 59
# speedup vs baseline: 1.0086x; 1.0086x over previous
"""Bahdanau attention Trainium2 kernel (nn_BahdanauAttn_52862457479409).

Shapes (hardcoded): B=64, S=4096, H=128, D=2H=256, f32.
Sharding: data-parallel over batch, 8 batches per core on 8 cores.

Per-core per-batch pipeline (SPMD over cores, software-pipelined so
phase B of batch b is emitted after the score-front of batch b+1):
  1. enc[b] (4 MB) loaded HBM->SBUF as float32r natural layout
     nat[p=s%128, c=s//128, d] in 16 split DMAs on the pure SP queue
     (the f32r-typed DMA delivers hardware-rounded data, which the
     fp32r matmul path requires anyway).
  2. PE-transposes (f32r, 1.5 cy/row) -> encT tiles [128d, 512s];
     PSUM->SBUF copies balanced ~11/5 across DVE/ACT.
  3. Phase A: enc_featT = WhT_r.T @ encT (f32r, N=512, full rate).
  4. ACT tanh with per-partition bias dec_featT[:, b] -> T tiles (f32).
  5. v-matmuls (fp32, exact) batched at front end: scoresT[128s, 32c]
     in PSUM - softmax over all 4096 is lane-parallel in this layout;
     max-subtraction is safely skipped (|scores| <= ~6).
  6. exp (ACT) + fused mask-select/row-sum (DVE scalar_tensor_tensor)
     -> em; cross-partition total + reciprocal broadcast via tiny PE
     ones-matmuls.
  7. attn output: PE transpose of normalized attnT -> [32, 128] rows.
  8. Phase B on UNNORMALIZED weights: 32 f32r matmuls with em column
     pairs stationary (padded to 2 cols for the even-count ISA rule,
     garbage row 1 ignored), rhs = nat chunks N=256; context scaled by
     1/total at the end and stored as one contiguous row.

Only the fp32r rounding (~11 mantissa bits) of enc/Wh/em leaves the
fp32 envelope; measured ~1.3e-4 relative error on both outputs.
TimelineSim: ~128 us/core against a ~97 us DMA roofline.
"""

from contextlib import ExitStack

import numpy as np

import concourse.bass as bass
import concourse.tile as tile
from concourse import bacc, mybir
from concourse.bass_utils import run_bass_kernel_spmd
from concourse.masks import make_identity

B, S, H = 64, 4096, 128
D = 2 * H
N_CORES = 8
BPC = B // N_CORES  # batches per core
C = S // 128  # 32 s-chunks of 128
NT = 8  # s-tiles per batch
TS = S // NT  # 512, s-tile size
CPT = TS // 128  # 4 chunks per tile

_CACHE = {}


def _build():
    if "nc" in _CACHE:
        return _CACHE["nc"]
    dt = mybir.dt
    nc = bacc.Bacc("TRN2", target_bir_lowering=False, debug=False, num_devices=N_CORES)

    enc = nc.dram_tensor("enc", [BPC, S, D], dt.float32, kind="ExternalInput").ap()
    msk = nc.dram_tensor("msk", [BPC, S], dt.uint8, kind="ExternalInput").ap()
    hid = nc.dram_tensor("hid", [BPC, H], dt.float32, kind="ExternalInput").ap()
    cel = nc.dram_tensor("cel", [BPC, H], dt.float32, kind="ExternalInput").ap()
    wh = nc.dram_tensor("wh", [H, D], dt.float32, kind="ExternalInput").ap()
    ws = nc.dram_tensor("ws", [H, D], dt.float32, kind="ExternalInput").ap()
    wsb = nc.dram_tensor("wsb", [H], dt.float32, kind="ExternalInput").ap()
    vw = nc.dram_tensor("vw", [1, H], dt.float32, kind="ExternalInput").ap()
    ctx_o = nc.dram_tensor("ctx", [BPC, D], dt.float32, kind="ExternalOutput").ap()
    attn_o = nc.dram_tensor("attn", [BPC, S], dt.float32, kind="ExternalOutput").ap()

    with tile.TileContext(nc) as tc, ExitStack() as ctx:
        singles = ctx.enter_context(tc.tile_pool(name="singles", bufs=1))
        natp = ctx.enter_context(tc.tile_pool(name="nat", bufs=3))
        etp = ctx.enter_context(tc.tile_pool(name="encT", bufs=4))
        ttp = ctx.enter_context(tc.tile_pool(name="tanh", bufs=12))
        smallp = ctx.enter_context(tc.tile_pool(name="small", bufs=4))
        # PSUM pools: 2 + 2 + 1 + 2 + 1 = 8 banks
        tpp = ctx.enter_context(tc.tile_pool(name="tp_ps", bufs=4, space="PSUM"))
        efp = ctx.enter_context(tc.tile_pool(name="ef_ps", bufs=1, space="PSUM"))
        scp = ctx.enter_context(tc.tile_pool(name="sc_ps", bufs=1, space="PSUM"))
        cxp = ctx.enter_context(tc.tile_pool(name="cx_ps", bufs=1, space="PSUM"))
        mip = ctx.enter_context(tc.tile_pool(name="mi_ps", bufs=1, space="PSUM"))

        ident = singles.tile([128, 128], dt.float32)
        make_identity(nc, ident[:])
        ident_r = singles.tile([128, 128], dt.float32r)
        nc.vector.tensor_copy(ident_r[:], ident[:])
        ones = singles.tile([128, 1], dt.float32)
        nc.vector.memset(ones[:], 1.0)
        ones_row = singles.tile([1, 128], dt.float32)
        nc.vector.memset(ones_row[:], 1.0)
        zeros2 = singles.tile([128, 2], dt.float32)
        nc.vector.memset(zeros2[:], 0.0)

        # ---- setup: WhT (f32r), WsT (f32), v column (f32r), dec_featT ----
        swh = singles.tile([H, D], dt.float32)
        nc.sync.dma_start(swh[:], wh[:])
        sws = singles.tile([H, D], dt.float32)
        nc.sync.dma_start(sws[:], ws[:])
        whT_r = singles.tile([128, 2, 128], dt.float32r)
        wsT = singles.tile([128, 2, 128], dt.float32)
        for half in range(2):
            pt = tpp.tile([128, 128], dt.float32, tag="tp")
            nc.tensor.transpose(pt[:], swh[:, bass.ts(half, 128)], ident[:])
            nc.vector.tensor_copy(whT_r[:, half, :], pt[:])
            pt2 = tpp.tile([128, 128], dt.float32, tag="tp")
            nc.tensor.transpose(pt2[:], sws[:, bass.ts(half, 128)], ident[:])
            nc.vector.tensor_copy(wsT[:, half, :], pt2[:])

        # v row -> column (PE transpose), rounded to f32r
        sv = singles.tile([1, H], dt.float32)
        nc.sync.dma_start(sv[:], vw[:])
        pv = mip.tile([128, 1], dt.float32, tag="mi")
        nc.tensor.transpose(pv[:], sv[:], ident[:1, :1])
        v_r = singles.tile([128, 1], dt.float32)
        nc.vector.tensor_copy(v_r[:], pv[:])

        # dec_featT [128h, BPC] = WsT.T @ [hidT; celT] + wsb
        shid = singles.tile([BPC, H], dt.float32)
        nc.sync.dma_start(shid[:], hid[:])
        scel = singles.tile([BPC, H], dt.float32)
        nc.sync.dma_start(scel[:], cel[:])
        sbias = singles.tile([H, 1], dt.float32)
        nc.sync.dma_start(sbias[:], wsb[:, None])
        hidT = singles.tile([H, BPC], dt.float32)
        celT = singles.tile([H, BPC], dt.float32)
        ph = mip.tile([H, BPC], dt.float32, tag="mi")
        nc.tensor.transpose(ph[:], shid[:], ident[:BPC, :BPC])
        nc.vector.tensor_copy(hidT[:], ph[:])
        pc2 = mip.tile([H, BPC], dt.float32, tag="mi")
        nc.tensor.transpose(pc2[:], scel[:], ident[:BPC, :BPC])
        nc.vector.tensor_copy(celT[:], pc2[:])
        pdf = mip.tile([H, BPC], dt.float32, tag="mi")
        nc.tensor.matmul(pdf[:], wsT[:, 0, :], hidT[:], start=True, stop=False)
        nc.tensor.matmul(pdf[:], wsT[:, 1, :], celT[:], start=False, stop=True)
        dec_featT = singles.tile([H, BPC], dt.float32)
        nc.scalar.activation(
            dec_featT[:], pdf[:], mybir.ActivationFunctionType.Identity, bias=sbias[:]
        )

        # ---- per-batch pipeline (software-pipelined emission) ----
        # front(b): DMA + transposes + phase A + tanh + v + softmax -> attnT
        # phaseB(b) is emitted AFTER front(b+1) so the in-order PE stream
        # always has ready work while batch b's softmax chain runs on DVE/ACT.
        state = {}

        def front(b):
            nat = natp.tile([128, C, D], dt.float32r)
            enc_b = enc[b].rearrange("(c p) d -> p c d", p=128).bitcast(dt.float32r)
            for k in range(16):
                nc.sync.dma_start(
                    nat[:, bass.ts(k, C // 16), :], enc_b[:, bass.ts(k, C // 16), :]
                )

            # mask [32, 128] natural -> f32 -> maskT [128, 32]
            mnat = smallp.tile([C, 128], dt.uint8)
            nc.gpsimd.dma_start(mnat[:], msk[b].rearrange("(c p) -> c p", p=128))
            mnat_f = smallp.tile([C, 128], dt.float32)
            nc.vector.tensor_copy(mnat_f[:], mnat[:])
            pmt = mip.tile([128, C], dt.float32, tag="mi")
            nc.tensor.transpose(pmt[:], mnat_f[:], ident[:C, :C])
            maskT = smallp.tile([128, C], dt.float32)
            nc.vector.tensor_copy(maskT[:], pmt[:])

            scoresT = scp.tile([128, C], dt.float32)
            tts = []
            for t in range(NT):
                eT0 = etp.tile([128, TS], dt.float32r, tag="eT0")
                eT1 = etp.tile([128, TS], dt.float32r, tag="eT1")
                for half, eT in ((0, eT0), (1, eT1)):
                    tp = tpp.tile([128, TS], dt.float32r, tag="tp")
                    for j in range(CPT):
                        ch = CPT * t + j
                        nc.tensor.transpose(
                            tp[:, bass.ts(j, 128)],
                            nat[:, ch, bass.ts(half, 128)],
                            ident_r[:],
                        )
                    # PSUM->SBUF copy; pair-parallel: half1 goes to ACT on
                    # 5 of 8 tiles so both halves of those tiles copy at once
                    if half == 1 and t % 8 not in (2, 5, 7):
                        nc.scalar.copy(eT[:], tp[:])
                    else:
                        nc.vector.tensor_copy(eT[:], tp[:])
                ef = efp.tile([128, TS], dt.float32)
                nc.tensor.matmul(ef[:], whT_r[:, 0, :], eT0[:], start=True, stop=False)
                nc.tensor.matmul(ef[:], whT_r[:, 1, :], eT1[:], start=False, stop=True)
                tt = ttp.tile([128, TS], dt.float32)
                nc.scalar.activation(
                    tt[:],
                    ef[:],
                    mybir.ActivationFunctionType.Tanh,
                    bias=dec_featT[:, b : b + 1],
                )
                tts.append(tt)
            for t in range(NT):
                tt = tts[t]
                for j in range(CPT):
                    nc.tensor.matmul(
                        scoresT[:, CPT * t + j : CPT * t + j + 1],
                        tt[:, bass.ts(j, 128)],
                        v_r[:],
                        start=True,
                        stop=True,
                    )

            # softmax over all 4096 (no max-subtraction: |scores| <= ~6)
            expT = smallp.tile([128, C], dt.float32)
            nc.scalar.activation(expT[:], scoresT[:], mybir.ActivationFunctionType.Exp)
            em = smallp.tile([128, C], dt.float32)
            partial = smallp.tile([128, 1], dt.float32)
            # em = (maskT == 0) * expT ; partial = row-sum(em)
            nc.vector.scalar_tensor_tensor(
                out=em[:],
                in0=maskT[:],
                scalar=0.0,
                in1=expT[:],
                op0=mybir.AluOpType.is_equal,
                op1=mybir.AluOpType.mult,
                accum_out=partial[:],
            )
            # phase B runs on UNNORMALIZED weights (em); ctx scaled at end.
            em_r = smallp.tile([128, C + 2], dt.float32r)
            nc.vector.tensor_copy(em_r[:, :C], em[:])
            nc.vector.tensor_copy(em_r[:, C : C + 2], zeros2[:])
            ptot = mip.tile([1, 1], dt.float32, tag="mi")
            nc.tensor.matmul(ptot[:], partial[:], ones[:], start=True, stop=True)
            recip = smallp.tile([1, 1], dt.float32)
            nc.vector.reciprocal(recip[:], ptot[:])
            recip_bc = mip.tile([128, 1], dt.float32, tag="mi")
            nc.tensor.matmul(recip_bc[:], ones_row[:], recip[:], start=True, stop=True)
            attnT = smallp.tile([128, C], dt.float32)
            nc.vector.tensor_scalar_mul(attnT[:], em[:], recip_bc[:])
            state[b] = (nat, attnT, em_r, recip)

        def phase_b(b):
            nat, attnT, em_r, recip = state.pop(b)
            # attn output: transpose -> [32, 128] natural rows
            pat = mip.tile([C, 128], dt.float32, tag="mi")
            nc.tensor.transpose(pat[:], attnT[:], ident[:])
            at_sb = smallp.tile([C, 128], dt.float32)
            nc.vector.tensor_copy(at_sb[:], pat[:])
            nc.gpsimd.dma_start(attn_o[b].rearrange("(c p) -> c p", p=128), at_sb[:])

            # phase B: context via attn-stationary f32r matmuls (row 1 garbage)
            cx = cxp.tile([2, D], dt.float32, tag="cx")
            for c in range(C):
                nc.tensor.matmul(
                    cx[:],
                    em_r[:, c : c + 2],
                    nat[:, c, :],
                    start=(c == 0),
                    stop=(c == C - 1),
                )
            ctx_sb = smallp.tile([1, D], dt.float32)
            nc.vector.tensor_scalar_mul(ctx_sb[:], cx[0:1, :], recip[:])
            nc.gpsimd.dma_start(ctx_o[b : b + 1, :], ctx_sb[:])

        for b in range(BPC):
            front(b)
            if b > 0:
                phase_b(b - 1)
        phase_b(BPC - 1)

    nc.compile()
    _CACHE["nc"] = nc
    return nc


def kernel(
    encoder_outputs,
    encoder_padding_mask,
    hidden_state,
    cell_state,
    Wh_w,
    Ws_w,
    Ws_b,
    v_w,
):
    nc = _build()
    enc = np.ascontiguousarray(np.asarray(encoder_outputs, dtype=np.float32))
    mask = np.asarray(encoder_padding_mask).astype(np.uint8)
    hid = np.asarray(hidden_state, dtype=np.float32)
    cel = np.asarray(cell_state, dtype=np.float32)
    wh = np.ascontiguousarray(np.asarray(Wh_w, dtype=np.float32))
    ws = np.ascontiguousarray(np.asarray(Ws_w, dtype=np.float32))
    wsb = np.asarray(Ws_b, dtype=np.float32)
    vw = np.ascontiguousarray(np.asarray(v_w, dtype=np.float32))

    in_maps = []
    for c in range(N_CORES):
        sl = slice(c * BPC, (c + 1) * BPC)
        in_maps.append(
            {
                "enc": enc[sl],
                "msk": mask[sl],
                "hid": hid[sl],
                "cel": cel[sl],
                "wh": wh,
                "ws": ws,
                "wsb": wsb,
                "vw": vw,
            }
        )
    res = run_bass_kernel_spmd(nc, in_maps, core_ids=list(range(N_CORES)))
    context = np.concatenate([r["ctx"] for r in res.results], axis=0)
    attn = np.concatenate([r["attn"] for r in res.results], axis=0)
    return context, attn
